# revision 15
# baseline (speedup 1.0000x reference)
"""Trainium2 Bass kernel for DecoupledIntraInterMultiSequenceSelfAttention.

Sharding: 8 cores = (batch in {0,1}) x (head-group in {0..3}, 4 heads each).
Each core computes both branches (intra/inter) for its batch/heads, plus the
partial output projection over its heads' columns of Wout. Host sums the 4
partial [D, L] outputs per batch and transposes back.

Device layout: feature-major ("transposed") throughout.
  - q,k produced as qkT [f, t] tiles (f on partitions), RoPE applied there.
  - scoresT[s, t] = k_aug.T @ q_aug per head (s on PSUM partitions).
  - em = exp(scoresT) -> bf16 sbuf; masked by 0/1 bf16 chain masks.
  - AV with fused softmax sums: stationary lhsT = [v_head | ones] (M=65),
    so avT[0:64] = sum_s v[s,d] em[s,t] and avT[64] = sum_s em[s,t] in the
    same matmul (per-head [65,512] PSUM bank).
  - normalize: reciprocal of the sums row, broadcast to 64 partitions via a
    K=1 matmul, per-head multiply -> attnT bf16 [d, t].
  - out projection: outT[e, t] += WoutT_chunk.T @ attnT_chunk, bias fused.
"""

import sys

sys.path.insert(0, "/opt/trn_rl_repo")

import numpy as np
import ml_dtypes

import concourse.bass as bass
import concourse.bacc as bacc
import concourse.tile as tile
from concourse import mybir
from concourse.bass_utils import run_bass_kernel_spmd

BF16 = ml_dtypes.bfloat16
LAST_SIM_NS = None
LAST_META = None
dt = mybir.dt
Alu = mybir.AluOpType
Act = mybir.ActivationFunctionType

B, L, D, H = 2, 2048, 1024, 16
HD = D // H
ROPE_BASE = 10000.0
NCORES = 8
HPC = 4  # heads per core
ST = L // 128  # 16 s-tiles
TC = L // 512  # 4 t-chunks
KC = D // 128  # 8 contraction chunks for projections


# ---------------------------------------------------------------- host math
def _chain_info_np(lengths_row):
    csum = np.cumsum(lengths_row.astype(np.int64))
    total = csum[-1]
    p = np.arange(L)
    cid = np.searchsorted(csum, p, side="right")
    prev = np.where(cid > 0, csum[np.clip(cid - 1, 0, L - 1)], 0)
    valid = p < total
    pos = np.where(valid, p - prev, 0).astype(np.float32)
    return pos, cid.astype(np.int64), valid


def _rope_tables_fm(pos):
    # feature-major cos/sin tables [128, L] (64-row pattern replicated x2),
    # with the rotation sign baked into sin ("sinS").
    inv_freq = 1.0 / (ROPE_BASE ** (np.arange(0, HD, 2, dtype=np.float32) / HD))
    # row d (0..63): frequency inv_freq[d % 32]
    freqs = pos[None, :] * inv_freq[(np.arange(HD) % (HD // 2)), None]  # [64, L]
    cos = np.cos(freqs)
    sin = np.sin(freqs)
    sinS = sin.copy()
    sinS[: HD // 2] *= -1.0  # rot(x)[d] = -x[d+32] for d<32 ; +x[d-32] else
    cos128 = np.concatenate([cos, cos], axis=0)
    sinS128 = np.concatenate([sinS, sinS], axis=0)
    return cos128.astype(BF16), sinS128.astype(BF16)


def _masks_np(cid, valid):
    cid_k = np.where(valid, cid, -5)  # sentinel for invalid keys
    eq = cid_k[:, None] == cid[None, :]  # [s, t] same-chain & key-valid
    intra = eq & valid[:, None]
    inter = valid[:, None] & ~eq
    return intra.astype(BF16), inter.astype(BF16)


# ------------------------------------------------------------- program build
def _build_program(meta, reps=1):
    """meta: dict with act[br][st][tc], mul[br][st][tc], expbias[br], eps[br][tc]"""
    act = meta["act"]
    mul = meta["mul"]
    expbias = meta["expbias"]
    need_eps = meta["eps"]

    nc = bacc.Bacc(None, target_bir_lowering=False)
    f32, bf = dt.float32, dt.bfloat16

    xt = nc.declare_dram_parameter("xt", [KC, 128, L], bf, isOutput=False)
    wqk = nc.declare_dram_parameter("wqk", [KC, 128, 1024], bf, isOutput=False)
    wv = nc.declare_dram_parameter("wv", [KC, 128, 512], bf, isOutput=False)
    wout = nc.declare_dram_parameter("wout", [2, 128, 1024], bf, isOutput=False)
    bqk = nc.declare_dram_parameter("bqk", [128, 8], f32, isOutput=False)
    bv = nc.declare_dram_parameter("bv", [128, 512], f32, isOutput=False)
    boutp = nc.declare_dram_parameter("bout_t", [128, 8], f32, isOutput=False)
    cosf = nc.declare_dram_parameter("cosf", [128, L], bf, isOutput=False)
    sinf = nc.declare_dram_parameter("sinf", [128, L], bf, isOutput=False)
    nmul = max(1, meta["nmul"])
    mask_d = nc.declare_dram_parameter("maskp", [nmul, 128, 512], bf, isOutput=False)
    outt = nc.declare_dram_parameter("outt", [8, 128, L], f32, isOutput=True)

    with tile.TileContext(nc) as tc:
        import contextlib

        ctx = contextlib.ExitStack()
        with ctx:
            persist = ctx.enter_context(tc.tile_pool(name="persist", bufs=1))
            work = ctx.enter_context(tc.tile_pool(name="work", bufs=2))
            empool = ctx.enter_context(tc.tile_pool(name="em", bufs=1))
            mpool = ctx.enter_context(tc.tile_pool(name="mask", bufs=8))
            pq_ctx = contextlib.ExitStack()
            pq = pq_ctx.enter_context(tc.tile_pool(name="pq", bufs=2, space="PSUM"))

            # ---- persistent sbuf tiles
            xt_sb = [persist.tile([128, L], bf, tag=f"xt{k}", name=f"xt{k}") for k in range(KC)]
            wqk_sb = [persist.tile([128, 1024], bf, tag=f"wqk{k}", name=f"wqk{k}") for k in range(KC)]
            wv_sb = [persist.tile([128, 512], bf, tag=f"wv{k}", name=f"wv{k}") for k in range(KC)]
            wout_sb = [persist.tile([128, 1024], bf, tag=f"wo{k}", name=f"wo{k}") for k in range(2)]
            cos_sb = persist.tile([128, L], bf, tag="cos", name="cos_sb")
            sin_sb = persist.tile([128, L], bf, tag="sin", name="sin_sb")
            bqk_sb = persist.tile([128, 8], f32, tag="bqk", name="bqk_sb")
            bo_sb = persist.tile([128, 8], f32, tag="bo", name="bo_sb")
            bv_bc = persist.tile([128, 512], f32, tag="bvb", name="bv_bc")
            ones512 = persist.tile([1, 512], bf, tag="ones512", name="ones512")
            epsv = persist.tile([1, 65], bf, tag="epsv", name="epsv")
            # per-head blocks of 65 cols: [v features (64) | ones] x 8 heads
            v_sb = [persist.tile([128, 520], bf, tag=f"v{s}", name=f"v{s}") for s in range(ST)]
            qkr = [persist.tile([128, L], bf, tag=f"qkr{i}", name=f"qkr{i}") for i in range(8)]
            attnT = [
                persist.tile([128, L], bf, tag=f"at{i}", name=f"at{i}") for i in range(4)
            ]  # [br*2+hp]

            for k in range(KC):
                (nc.scalar if k % 2 else nc.sync).dma_start(xt_sb[k][:], xt[k])
                (nc.sync if k % 2 else nc.scalar).dma_start(wqk_sb[k][:], wqk[k])
                nc.gpsimd.dma_start(wv_sb[k][:], wv[k])
            for k in range(2):
                nc.gpsimd.dma_start(wout_sb[k][:], wout[k])
            nc.gpsimd.dma_start(cos_sb[:], cosf[:])
            nc.gpsimd.dma_start(sin_sb[:], sinf[:])
            nc.gpsimd.dma_start(bqk_sb[:], bqk[:])
            nc.gpsimd.dma_start(bo_sb[:], boutp[:])
            nc.gpsimd.dma_start(bv_bc[:], bv[:])
            nc.vector.memset(ones512[:], 1.0)
            nc.vector.memset(epsv[:], 0.0)
            nc.vector.memset(epsv[:, 64:65], 1e-30)
            for st in range(ST):
                # ones column per head block (col 64 of each 65-wide block)
                nc.gpsimd.memset(
                    v_sb[st][:].rearrange("p (g c) -> p g c", g=8)[:, :, 64:65], 1.0
                )

            # shared psum pool for attention + late projections (created after
            # the early-phase pq pool is released; budget: av 4 + sc 3 + fill 1 = 8)
            pat_ctx = contextlib.ExitStack()

            def _rope(ft, qk_t):
                # RoPE: qkr = qk*cos + rot(qk)*sinS  (rot = partition swap +-32)
                rot = work.tile([128, L], bf, tag="rot", bufs=2, name=f"rot{ft}")
                for h in range(4):
                    src_p = (h ^ 1) * 32
                    nc.vector.tensor_copy(
                        rot[h * 32 : h * 32 + 32, :], qk_t[src_p : src_p + 32, :]
                    )
                tmp = work.tile([128, L], bf, tag="ropetmp", bufs=2, name=f"rtmp{ft}")
                nc.vector.tensor_tensor(tmp[:], qk_t[:], cos_sb[:], Alu.mult)
                nc.gpsimd.tensor_tensor(rot[:], rot[:], sin_sb[:], Alu.mult)
                nc.vector.tensor_tensor(qkr[ft][:], tmp[:], rot[:], Alu.add)

            def _qk_chunk(ft, tp, grp, qk_t, psum_pool, ptag, bufs):
                pr = [
                    psum_pool.tile(
                        [128, 512], f32, tag=ptag, bufs=bufs,
                        name=f"qkps{ft}_{tp}_{j}",
                    )
                    for j in range(grp)
                ]
                for k in range(KC):
                    for j in range(grp):
                        tcc = tp * grp + j
                        nc.tensor.matmul(
                            pr[j][:],
                            wqk_sb[k][:, ft * 128 : (ft + 1) * 128],
                            xt_sb[k][:, tcc * 512 : (tcc + 1) * 512],
                            start=(k == 0),
                            stop=(k == KC - 1),
                        )
                for j in range(grp):
                    tcc = tp * grp + j
                    nc.vector.tensor_scalar(
                        qk_t[:, tcc * 512 : (tcc + 1) * 512],
                        pr[j][:],
                        bqk_sb[:, ft : ft + 1],
                        None,
                        Alu.add,
                    )

            def qk_proj(ft, psum_pool, ptag):
                # tcc-paired in the 2-slot early pool so each ldweights(wqk
                # chunk) serves 2 matmuls; single-slot pools go tcc-sequential.
                qk_t = work.tile([128, L], bf, tag="qk", bufs=2, name=f"qk{ft}")
                grp = 2 if ptag == "pqs" else 1
                bufs = 2
                for tp in range(TC // grp):
                    _qk_chunk(ft, tp, grp, qk_t, psum_pool, ptag, bufs)
                _rope(ft, qk_t)

            def qk_proj_tasks(ft, psum_pool, ptag):
                # fill-task closures: one tcc per task + a final rope task
                qk_t = work.tile([128, L], bf, tag="qk", bufs=2, name=f"qk{ft}")
                tasks = [
                    (lambda ft=ft, tp=tp, qk_t=qk_t: _qk_chunk(
                        ft, tp, 1, qk_t, psum_pool, ptag, 1))
                    for tp in range(TC)
                ]
                tasks.append(lambda ft=ft, qk_t=qk_t: _rope(ft, qk_t))
                return tasks

            def _v_chunk(st, br, psum_pool, ptag, bufs):
                ps = psum_pool.tile(
                    [128, 256], f32, tag=ptag, bufs=bufs, name=f"vps{br}_{st}"
                )
                for k in range(KC):
                    nc.tensor.matmul(
                        ps[:],
                        xt_sb[k][:, st * 128 : (st + 1) * 128],
                        wv_sb[k][:, br * 256 : (br + 1) * 256],
                        start=(k == 0),
                        stop=(k == KC - 1),
                    )
                vout = v_sb[st][:, br * 260 : (br + 1) * 260].rearrange(
                    "p (g c) -> p g c", g=4
                )[:, :, 0:64]
                nc.vector.scalar_tensor_tensor(
                    vout,
                    ps[:].rearrange("p (g c) -> p g c", g=4),
                    0.0,
                    bv_bc[:, br * 256 : (br + 1) * 256].rearrange("p (g c) -> p g c", g=4),
                    Alu.bypass,
                    Alu.add,
                )

            def v_proj_tasks(br, psum_pool, ptag):
                return [
                    (lambda st=st: _v_chunk(st, br, psum_pool, ptag, 1))
                    for st in range(ST)
                ]

            fillq = []

            def pop_fill(n=1):
                for _ in range(n):
                    if fillq:
                        fillq.pop(0)()

            def drain_fill():
                while fillq:
                    fillq.pop(0)()

            def outproj_tile(et, tcc):
                po = pat_holder[0].tile(
                    [128, 512], f32, tag="fill", bufs=1, name=f"po{et}_{tcc}",
                )
                n = 0
                for br in range(2):
                    for hp in range(2):
                        nc.tensor.matmul(
                            po[:],
                            wout_sb[hp][:, et * 128 : (et + 1) * 128],
                            attnT[br * 2 + hp][:, tcc * 512 : (tcc + 1) * 512],
                            start=(n == 0),
                            stop=(n == 3),
                        )
                        n += 1
                osb = work.tile([128, 512], f32, tag="osb", bufs=3, name=f"osb{et}_{tcc}")
                nc.vector.tensor_scalar(
                    osb[:], po[:], bo_sb[:, et : et + 1], None, Alu.add
                )
                nc.sync.dma_start(outt[et, :, tcc * 512 : (tcc + 1) * 512], osb[:])

            mulctr = [0]
            mask_idx = {}

            def attention(br, pat, tcs, emit_po=False):
                for tcc in tcs:
                    asts = [st for st in range(ST) if act[br][st][tcc]]
                    if not asts:
                        for hp in range(2):
                            nc.vector.memset(
                                attnT[br * 2 + hp][:, tcc * 512 : (tcc + 1) * 512], 0.0
                            )
                        continue
                    eps_here = need_eps[br][tcc]
                    # per-head [65, 512] AV banks: rows 0:64 = features, 64 = sum
                    avs = [
                        pat.tile([65, 512], f32, tag=f"av{g}", bufs=1, name=f"av{br}_{g}_{tcc}")
                        for g in range(4)
                    ]
                    for st in asts:
                        pop_fill(1 + (len(fillq) > 20))
                        first = st == asts[0]
                        last = st == asts[-1]
                        needmul = mul[br][st][tcc]
                        mk = None
                        if needmul:
                            key = (br, tcc, st)
                            if key not in mask_idx:
                                mask_idx[key] = mulctr[0]
                                mulctr[0] += 1
                            mk = mpool.tile(
                                [128, 512], bf, tag="mk", name=f"mk{br}_{st}_{tcc}"
                            )
                            nc.sync.dma_start(mk[:], mask_d[mask_idx[key]])
                        for hp in range(2):
                            qf = qkr[4 * br + hp]
                            kf = qkr[4 * br + 2 + hp]
                            for ab in range(2):
                                g = hp * 2 + ab
                                sc = pat.tile(
                                    [128, 512], f32, tag="sc", bufs=3,
                                    name=f"sc{br}_{g}_{st}_{tcc}",
                                )
                                nc.tensor.matmul(
                                    sc[:],
                                    kf[ab * 64 : ab * 64 + 64, st * 128 : (st + 1) * 128],
                                    qf[ab * 64 : ab * 64 + 64, tcc * 512 : (tcc + 1) * 512],
                                    start=True,
                                    stop=True,
                                    tile_position=(ab * 64, 0),
                                )
                                em = empool.tile(
                                    [128, 512], bf, tag="em", bufs=6,
                                    name=f"em{br}_{g}_{st}_{tcc}",
                                )
                                nc.scalar.activation(em[:], sc[:], Act.Exp, bias=expbias[br])
                                if needmul:
                                    late = br == 1 and tcc == TC - 1
                                    eng = nc.vector if (late or (st + g) % 2) else nc.gpsimd
                                    eng.tensor_tensor(em[:], em[:], mk[:], Alu.mult)
                                nc.tensor.matmul(
                                    avs[g][:],
                                    v_sb[st][:, (br * 4 + g) * 65 : (br * 4 + g) * 65 + 65],
                                    em[:],
                                    start=first,
                                    stop=(last and not eps_here),
                                    tile_position=(0, 0),
                                )
                    if eps_here:
                        for g in range(4):
                            nc.tensor.matmul(
                                avs[g][:],
                                epsv[:],
                                ones512[:],
                                start=False,
                                stop=True,
                                tile_position=(0, 0),
                            )
                    # normalize: attnT = av[0:64] * (1/av[64]) broadcast to 64 rows
                    # (gpsimd can't touch PSUM; DVE TT allows only one PSUM input,
                    # so broadcast 1/sums into SBUF on Pool, multiply on DVE)
                    for hp in range(2):
                        for ab in range(2):
                            g = hp * 2 + ab
                            rcp = work.tile(
                                [1, 512], f32, tag="rcp", bufs=4,
                                name=f"rcp{br}_{g}_{tcc}",
                            )
                            nc.vector.reciprocal(rcp[:], avs[g][64:65, :])
                            rb_sb = work.tile(
                                [64, 512], f32, tag="rb", bufs=4, name=f"rb{br}_{g}_{tcc}"
                            )
                            nc.gpsimd.partition_broadcast(rb_sb[:], rcp[:])
                            nc.vector.tensor_tensor(
                                attnT[br * 2 + hp][
                                    ab * 64 : ab * 64 + 64, tcc * 512 : (tcc + 1) * 512
                                ],
                                avs[g][0:64, :],
                                rb_sb[:],
                                Alu.mult,
                            )
                    if emit_po:
                        for et in range(8):
                            fillq.append(lambda et=et, tcc=tcc: outproj_tile(et, tcc))

            # ---- emission order chosen for cross-phase engine overlap:
            # k/q intra proj+rope and v_intra first (PE), then intra attention
            # (ACT-heavy) while inter projections fill PE, then inter attention
            # overlapping the output projection.
            pat_holder = [None]
            for _rep in range(reps):
                if _rep == 0:
                    # all inter-branch q/k eagerly (attention(1) reads them at
                    # its first st iteration with st-outer loops)
                    for ft in (6, 4, 7, 5):
                        qk_proj(ft, pq, "pqs")
                    pq_ctx.close()  # release early psum banks
                    pat_holder[0] = pat_ctx.enter_context(
                        tc.tile_pool(name="pat", bufs=1, space="PSUM")
                    )
                else:
                    for ft in (6, 4, 7, 5):
                        qk_proj(ft, pat_holder[0], "fill")
                pat = pat_holder[0]
                # braid the remaining projections into the ACT-heavy inter
                # branch; v1 first (st-ordered: AV(st) reads it 1 pop later).
                fillq.extend(v_proj_tasks(1, pat, "fill"))
                for ft in (2, 0, 3, 1):
                    fillq.extend(qk_proj_tasks(ft, pat, "fill"))
                fillq.extend(v_proj_tasks(0, pat, "fill"))
                attention(1, pat, list(range(TC)))
                drain_fill()  # all qkr/v writes emitted before intra readers
                attention(0, pat, list(range(TC)), emit_po=True)
                mask_idx.clear()
                mulctr[0] = 0

            drain_fill()
            pat_ctx.close()

    nc.finalize()
    return nc


# ------------------------------------------------------------------- kernel
def _kernel_prep(**inputs):
    hs = np.asarray(inputs["hidden_states"], np.float32)
    lens = np.asarray(inputs["attention_mask_in_length"])
    Wqkv = [np.asarray(inputs["Wqkv_intra"], np.float32), np.asarray(inputs["Wqkv_inter"], np.float32)]
    bqkv = [np.asarray(inputs["bqkv_intra"], np.float32), np.asarray(inputs["bqkv_inter"], np.float32)]
    Wout = np.asarray(inputs["Wout"], np.float32)
    bout = np.asarray(inputs["bout"], np.float32)

    # chain info + masks per batch
    pos_b, masks_b, rope_b = [], [], []
    for b in range(B):
        pos, cid, valid = _chain_info_np(lens[b])
        mi, mx = _masks_np(cid, valid)
        masks_b.append((mi, mx))
        rope_b.append(_rope_tables_fm(pos))

    # union tile activity across batches (single SPMD program)
    act = [[[False] * TC for _ in range(ST)] for _ in range(2)]
    mul = [[[False] * TC for _ in range(ST)] for _ in range(2)]
    eps = [[False] * TC for _ in range(2)]
    for br in range(2):
        for st in range(ST):
            for tcc in range(TC):
                for b in range(B):
                    m = masks_b[b][br][st * 128 : (st + 1) * 128, tcc * 512 : (tcc + 1) * 512]
                    a = bool(m.any())
                    act[br][st][tcc] |= a
                    mul[br][st][tcc] |= not bool(m.all())
        for tcc in range(TC):
            for b in range(B):
                col = masks_b[b][br][:, tcc * 512 : (tcc + 1) * 512].astype(np.float32)
                eps[br][tcc] |= bool((col.sum(axis=0) == 0).any())

    # exp-overflow guard: estimate score bound from a sample of positions
    expbias = [0.0, 0.0]
    idx = np.linspace(0, L - 1, 128).astype(int)
    for br in range(2):
        mx = 0.0
        for b in range(B):
            xs = hs[b][idx]  # [128, D]
            qkv = xs @ Wqkv[br].T + bqkv[br]
            q = qkv[:, :D].reshape(128, H, HD) * (HD**-0.5)
            k = qkv[:, D : 2 * D].reshape(128, H, HD)
            s = np.einsum("thd,shd->hts", q, k)
            mx = max(mx, float(np.abs(s).max()))
        if mx * 2.0 > 60.0:
            expbias[br] = -mx * 1.5  # RoPE preserves norms; 1.5x margin

    packs = [[], []]  # per batch; order mirrors device emission (first-use)
    for br in (1, 0):
        for tcc in range(TC):
            for st in range(ST):
                if act[br][st][tcc] and mul[br][st][tcc]:
                    for b in range(B):
                        packs[b].append(
                            masks_b[b][br][st * 128 : (st + 1) * 128, tcc * 512 : (tcc + 1) * 512]
                        )
    nmul = len(packs[0])
    maskp_b = [
        np.stack(p).astype(BF16) if p else np.zeros((1, 128, 512), BF16) for p in packs
    ]

    meta = {"act": act, "mul": mul, "expbias": expbias, "nmul": nmul, "eps": eps}
    global LAST_META
    LAST_META = meta
    nc = _build_program(meta)
    return nc, _in_maps(inputs, masks_b, rope_b, maskp_b)


def _in_maps(inputs, masks_b, rope_b, maskp_b):
    hs = np.asarray(inputs["hidden_states"], np.float32)
    Wqkv = [np.asarray(inputs["Wqkv_intra"], np.float32), np.asarray(inputs["Wqkv_inter"], np.float32)]
    bqkv = [np.asarray(inputs["bqkv_intra"], np.float32), np.asarray(inputs["bqkv_inter"], np.float32)]
    Wout = np.asarray(inputs["Wout"], np.float32)
    bout = np.asarray(inputs["bout"], np.float32)

    in_maps = []
    for c in range(NCORES):
        b, g = divmod(c, 4)
        h0 = g * HPC
        qrows = lambda br: Wqkv[br][h0 * HD : (h0 + HPC) * HD]
        krows = lambda br: Wqkv[br][D + h0 * HD : D + (h0 + HPC) * HD]
        vrows = lambda br: Wqkv[br][2 * D + h0 * HD : 2 * D + (h0 + HPC) * HD]
        qb = lambda br: bqkv[br][h0 * HD : (h0 + HPC) * HD]
        kb = lambda br: bqkv[br][D + h0 * HD : D + (h0 + HPC) * HD]
        vb = lambda br: bqkv[br][2 * D + h0 * HD : 2 * D + (h0 + HPC) * HD]
        sc = HD**-0.5
        # [D_in, 1024]: q_intra(256,scaled) k_intra(256) q_inter k_inter
        wqk_full = np.concatenate(
            [qrows(0).T * sc, krows(0).T, qrows(1).T * sc, krows(1).T], axis=1
        )
        wv_full = np.concatenate([vrows(0).T, vrows(1).T], axis=1)  # [D_in, 512]
        bqk_full = np.concatenate([qb(0) * sc, kb(0), qb(1) * sc, kb(1)])  # [1024]
        bv_full = np.concatenate([vb(0), vb(1)])  # [512]
        woutT = Wout[:, h0 * HD : (h0 + HPC) * HD].T  # [256, 1024]
        bout_c = bout if g == 0 else np.zeros_like(bout)
        cos128, sinS128 = rope_b[b]
        in_maps.append(
            {
                "xt": np.ascontiguousarray(
                    hs[b].T.reshape(KC, 128, L)
                ).astype(BF16),
                "wqk": np.ascontiguousarray(wqk_full.reshape(KC, 128, 1024)).astype(BF16),
                "wv": np.ascontiguousarray(wv_full.reshape(KC, 128, 512)).astype(BF16),
                "wout": np.ascontiguousarray(woutT.reshape(2, 128, 1024)).astype(BF16),
                "bqk": np.ascontiguousarray(bqk_full.reshape(8, 128).T).astype(np.float32),
                "bv": np.broadcast_to(bv_full, (128, 512)).astype(np.float32),
                "bout_t": np.ascontiguousarray(bout_c.reshape(8, 128).T).astype(np.float32),
                "cosf": np.ascontiguousarray(cos128),
                "sinf": np.ascontiguousarray(sinS128),
                "maskp": maskp_b[b],
            }
        )

    return in_maps


def prepare(**inputs):
    """Build the specialized program and per-core inputs."""
    return _kernel_prep(**inputs)


def kernel(**inputs):
    nc, in_maps = _kernel_prep(**inputs)
    try:  # cost-model estimate of HW time (NTFF profiling unavailable via axon)
        from concourse.bass_interp import CoreSim

        _sim = CoreSim(nc, no_exec=True, publish_trace=False)
        _sim.event_loop()
        global LAST_SIM_NS
        LAST_SIM_NS = _sim.time
    except Exception:
        LAST_SIM_NS = None

    res = run_bass_kernel_spmd(nc, in_maps, list(range(NCORES)))

    out = np.zeros((B, L, D), np.float32)
    for c in range(NCORES):
        b = c // 4
        ot = res.results[c]["outt"].reshape(D, L)  # [e, t]
        out[b] += ot.T
    return out


if __name__ == "__main__":
    rng = np.random.default_rng(0)
    import reference

    inputs = {k: np.asarray(v) for k, v in reference.setup_inputs().items()}
    got = kernel(**inputs)
    exp = np.asarray(reference.reference(**inputs))
    err = np.abs(got - exp).max() / np.abs(exp).max()
    print("rel err", err)



# revision 17
# speedup vs baseline: 1.0204x; 1.0204x over previous
"""Trainium2 Bass kernel for DecoupledIntraInterMultiSequenceSelfAttention.

Sharding: 8 cores = (batch in {0,1}) x (head-group in {0..3}, 4 heads each).
Each core computes both branches (intra/inter) for its batch/heads, plus the
partial output projection over its heads' columns of Wout. Host sums the 4
partial [D, L] outputs per batch and transposes back.

Device layout: feature-major ("transposed") throughout.
  - q,k produced as qkT [f, t] tiles (f on partitions), RoPE applied there.
  - scoresT[s, t] = k_aug.T @ q_aug per head (s on PSUM partitions).
  - em = exp(scoresT) -> bf16 sbuf; masked by 0/1 bf16 chain masks.
  - AV with fused softmax sums: stationary lhsT = [v_head | ones] (M=65),
    so avT[0:64] = sum_s v[s,d] em[s,t] and avT[64] = sum_s em[s,t] in the
    same matmul (per-head [65,512] PSUM bank).
  - normalize: reciprocal of the sums row, broadcast to 64 partitions via a
    K=1 matmul, per-head multiply -> attnT bf16 [d, t].
  - out projection: outT[e, t] += WoutT_chunk.T @ attnT_chunk, bias fused.
"""

import sys

sys.path.insert(0, "/opt/trn_rl_repo")

import numpy as np
import ml_dtypes

import concourse.bass as bass
import concourse.bacc as bacc
import concourse.tile as tile
from concourse import mybir
from concourse.bass_utils import run_bass_kernel_spmd

BF16 = ml_dtypes.bfloat16
LAST_SIM_NS = None
LAST_META = None
dt = mybir.dt
Alu = mybir.AluOpType
Act = mybir.ActivationFunctionType

B, L, D, H = 2, 2048, 1024, 16
HD = D // H
ROPE_BASE = 10000.0
NCORES = 8
HPC = 4  # heads per core
ST = L // 128  # 16 s-tiles
TC = L // 512  # 4 t-chunks
KC = D // 128  # 8 contraction chunks for projections


# ---------------------------------------------------------------- host math
def _chain_info_np(lengths_row):
    csum = np.cumsum(lengths_row.astype(np.int64))
    total = csum[-1]
    p = np.arange(L)
    cid = np.searchsorted(csum, p, side="right")
    prev = np.where(cid > 0, csum[np.clip(cid - 1, 0, L - 1)], 0)
    valid = p < total
    pos = np.where(valid, p - prev, 0).astype(np.float32)
    return pos, cid.astype(np.int64), valid


def _rope_tables_fm(pos):
    # feature-major cos/sin tables [128, L] (64-row pattern replicated x2),
    # with the rotation sign baked into sin ("sinS").
    inv_freq = 1.0 / (ROPE_BASE ** (np.arange(0, HD, 2, dtype=np.float32) / HD))
    # row d (0..63): frequency inv_freq[d % 32]
    freqs = pos[None, :] * inv_freq[(np.arange(HD) % (HD // 2)), None]  # [64, L]
    cos = np.cos(freqs)
    sin = np.sin(freqs)
    sinS = sin.copy()
    sinS[: HD // 2] *= -1.0  # rot(x)[d] = -x[d+32] for d<32 ; +x[d-32] else
    cos128 = np.concatenate([cos, cos], axis=0)
    sinS128 = np.concatenate([sinS, sinS], axis=0)
    return cos128.astype(BF16), sinS128.astype(BF16)


def _masks_np(cid, valid):
    cid_k = np.where(valid, cid, -5)  # sentinel for invalid keys
    eq = cid_k[:, None] == cid[None, :]  # [s, t] same-chain & key-valid
    intra = eq & valid[:, None]
    inter = valid[:, None] & ~eq
    return intra.astype(BF16), inter.astype(BF16)


# ------------------------------------------------------------- program build
def _build_program(meta, reps=1):
    """meta: dict with act[br][st][tc], mul[br][st][tc], expbias[br], eps[br][tc]"""
    act = meta["act"]
    mul = meta["mul"]
    expbias = meta["expbias"]
    need_eps = meta["eps"]

    nc = bacc.Bacc(None, target_bir_lowering=False)
    f32, bf = dt.float32, dt.bfloat16

    xt = nc.declare_dram_parameter("xt", [KC, 128, L], bf, isOutput=False)
    wqk = nc.declare_dram_parameter("wqk", [KC, 128, 1024], bf, isOutput=False)
    wv = nc.declare_dram_parameter("wv", [KC, 128, 512], bf, isOutput=False)
    wout = nc.declare_dram_parameter("wout", [2, 128, 1024], bf, isOutput=False)
    bqk = nc.declare_dram_parameter("bqk", [128, 8], f32, isOutput=False)
    bv = nc.declare_dram_parameter("bv", [128, 512], f32, isOutput=False)
    boutp = nc.declare_dram_parameter("bout_t", [128, 8], f32, isOutput=False)
    cosf = nc.declare_dram_parameter("cosf", [128, L], bf, isOutput=False)
    sinf = nc.declare_dram_parameter("sinf", [128, L], bf, isOutput=False)
    nmul = max(1, meta["nmul"])
    mask_d = nc.declare_dram_parameter("maskp", [nmul, 128, 512], bf, isOutput=False)
    outt = nc.declare_dram_parameter("outt", [8, 128, L], f32, isOutput=True)

    with tile.TileContext(nc) as tc:
        import contextlib

        ctx = contextlib.ExitStack()
        with ctx:
            persist = ctx.enter_context(tc.tile_pool(name="persist", bufs=1))
            work = ctx.enter_context(tc.tile_pool(name="work", bufs=2))
            empool = ctx.enter_context(tc.tile_pool(name="em", bufs=1))
            mpool = ctx.enter_context(tc.tile_pool(name="mask", bufs=8))
            pq_ctx = contextlib.ExitStack()
            pq = pq_ctx.enter_context(tc.tile_pool(name="pq", bufs=2, space="PSUM"))

            # ---- persistent sbuf tiles
            xt_sb = [persist.tile([128, L], bf, tag=f"xt{k}", name=f"xt{k}") for k in range(KC)]
            wqk_sb = [persist.tile([128, 1024], bf, tag=f"wqk{k}", name=f"wqk{k}") for k in range(KC)]
            wv_sb = [persist.tile([128, 512], bf, tag=f"wv{k}", name=f"wv{k}") for k in range(KC)]
            wout_sb = [persist.tile([128, 1024], bf, tag=f"wo{k}", name=f"wo{k}") for k in range(2)]
            cos_sb = persist.tile([128, L], bf, tag="cos", name="cos_sb")
            sin_sb = persist.tile([128, L], bf, tag="sin", name="sin_sb")
            bqk_sb = persist.tile([128, 8], f32, tag="bqk", name="bqk_sb")
            bo_sb = persist.tile([128, 8], f32, tag="bo", name="bo_sb")
            bv_bc = persist.tile([128, 512], f32, tag="bvb", name="bv_bc")
            ones512 = persist.tile([1, 512], bf, tag="ones512", name="ones512")
            epsv = persist.tile([1, 65], bf, tag="epsv", name="epsv")
            # per-head blocks of 65 cols: [v features (64) | ones] x 8 heads
            v_sb = [persist.tile([128, 520], bf, tag=f"v{s}", name=f"v{s}") for s in range(ST)]
            qkr = [persist.tile([128, L], bf, tag=f"qkr{i}", name=f"qkr{i}") for i in range(8)]
            attnT = [
                persist.tile([128, L], bf, tag=f"at{i}", name=f"at{i}") for i in range(4)
            ]  # [br*2+hp]

            for k in range(KC):
                (nc.scalar if k % 2 else nc.sync).dma_start(xt_sb[k][:], xt[k])
                (nc.sync if k % 2 else nc.scalar).dma_start(wqk_sb[k][:], wqk[k])
                nc.gpsimd.dma_start(wv_sb[k][:], wv[k])
            for k in range(2):
                nc.gpsimd.dma_start(wout_sb[k][:], wout[k])
            nc.gpsimd.dma_start(cos_sb[:], cosf[:])
            nc.gpsimd.dma_start(sin_sb[:], sinf[:])
            nc.gpsimd.dma_start(bqk_sb[:], bqk[:])
            nc.gpsimd.dma_start(bo_sb[:], boutp[:])
            nc.gpsimd.dma_start(bv_bc[:], bv[:])
            nc.vector.memset(ones512[:], 1.0)
            nc.vector.memset(epsv[:], 0.0)
            nc.vector.memset(epsv[:, 64:65], 1e-30)
            for st in range(ST):
                # ones column per head block (col 64 of each 65-wide block)
                nc.gpsimd.memset(
                    v_sb[st][:].rearrange("p (g c) -> p g c", g=8)[:, :, 64:65], 1.0
                )

            # shared psum pool for attention + late projections (created after
            # the early-phase pq pool is released; budget: scp 2x2 + av 2 + fill 2 = 8)
            pat_ctx = contextlib.ExitStack()

            def _rope(ft, qk_t):
                # RoPE: qkr = qk*cos + rot(qk)*sinS  (rot = partition swap +-32)
                rot = work.tile([128, L], bf, tag="rot", bufs=2, name=f"rot{ft}")
                for h in range(4):
                    src_p = (h ^ 1) * 32
                    nc.vector.tensor_copy(
                        rot[h * 32 : h * 32 + 32, :], qk_t[src_p : src_p + 32, :]
                    )
                tmp = work.tile([128, L], bf, tag="ropetmp", bufs=2, name=f"rtmp{ft}")
                nc.vector.tensor_tensor(tmp[:], qk_t[:], cos_sb[:], Alu.mult)
                nc.gpsimd.tensor_tensor(rot[:], rot[:], sin_sb[:], Alu.mult)
                nc.vector.tensor_tensor(qkr[ft][:], tmp[:], rot[:], Alu.add)

            def _qk_chunk(ft, tp, grp, qk_t, psum_pool, ptag, bufs):
                pr = [
                    psum_pool.tile(
                        [128, 512], f32, tag=ptag, bufs=bufs,
                        name=f"qkps{ft}_{tp}_{j}",
                    )
                    for j in range(grp)
                ]
                for k in range(KC):
                    for j in range(grp):
                        tcc = tp * grp + j
                        nc.tensor.matmul(
                            pr[j][:],
                            wqk_sb[k][:, ft * 128 : (ft + 1) * 128],
                            xt_sb[k][:, tcc * 512 : (tcc + 1) * 512],
                            start=(k == 0),
                            stop=(k == KC - 1),
                        )
                for j in range(grp):
                    tcc = tp * grp + j
                    nc.vector.tensor_scalar(
                        qk_t[:, tcc * 512 : (tcc + 1) * 512],
                        pr[j][:],
                        bqk_sb[:, ft : ft + 1],
                        None,
                        Alu.add,
                    )

            def qk_proj(ft, psum_pool, ptag):
                # tcc-paired in the 2-slot early pool so each ldweights(wqk
                # chunk) serves 2 matmuls; single-slot pools go tcc-sequential.
                qk_t = work.tile([128, L], bf, tag="qk", bufs=2, name=f"qk{ft}")
                grp = 2 if ptag == "pqs" else 1
                bufs = 2
                for tp in range(TC // grp):
                    _qk_chunk(ft, tp, grp, qk_t, psum_pool, ptag, bufs)
                _rope(ft, qk_t)

            def qk_proj_tasks(ft, psum_pool, ptag):
                # fill-task closures: one tcc per task + a final rope task
                qk_t = work.tile([128, L], bf, tag="qk", bufs=2, name=f"qk{ft}")
                tasks = [
                    (lambda ft=ft, tp=tp, qk_t=qk_t: _qk_chunk(
                        ft, tp, 1, qk_t, psum_pool, ptag, 2))
                    for tp in range(TC)
                ]
                tasks.append(lambda ft=ft, qk_t=qk_t: _rope(ft, qk_t))
                return tasks

            def _v_chunk(st, br, psum_pool, ptag, bufs):
                ps = psum_pool.tile(
                    [128, 256], f32, tag=ptag, bufs=bufs, name=f"vps{br}_{st}"
                )
                for k in range(KC):
                    nc.tensor.matmul(
                        ps[:],
                        xt_sb[k][:, st * 128 : (st + 1) * 128],
                        wv_sb[k][:, br * 256 : (br + 1) * 256],
                        start=(k == 0),
                        stop=(k == KC - 1),
                    )
                vout = v_sb[st][:, br * 260 : (br + 1) * 260].rearrange(
                    "p (g c) -> p g c", g=4
                )[:, :, 0:64]
                nc.vector.scalar_tensor_tensor(
                    vout,
                    ps[:].rearrange("p (g c) -> p g c", g=4),
                    0.0,
                    bv_bc[:, br * 256 : (br + 1) * 256].rearrange("p (g c) -> p g c", g=4),
                    Alu.bypass,
                    Alu.add,
                )

            def v_proj_tasks(br, psum_pool, ptag):
                return [
                    (lambda st=st: _v_chunk(st, br, psum_pool, ptag, 2))
                    for st in range(ST)
                ]

            fillq = []

            def pop_fill(n=1):
                for _ in range(n):
                    if fillq:
                        fillq.pop(0)()

            def drain_fill():
                while fillq:
                    fillq.pop(0)()

            def outproj_tile(et, tcc):
                po = pat_holder[0].tile(
                    [128, 512], f32, tag="fill", bufs=2, name=f"po{et}_{tcc}",
                )
                n = 0
                for br in range(2):
                    for hp in range(2):
                        nc.tensor.matmul(
                            po[:],
                            wout_sb[hp][:, et * 128 : (et + 1) * 128],
                            attnT[br * 2 + hp][:, tcc * 512 : (tcc + 1) * 512],
                            start=(n == 0),
                            stop=(n == 3),
                        )
                        n += 1
                osb = work.tile([128, 512], f32, tag="osb", bufs=3, name=f"osb{et}_{tcc}")
                nc.vector.tensor_scalar(
                    osb[:], po[:], bo_sb[:, et : et + 1], None, Alu.add
                )
                nc.sync.dma_start(outt[et, :, tcc * 512 : (tcc + 1) * 512], osb[:])

            mulctr = [0]
            mask_idx = {}

            def attention(br, pat, tcs, emit_po=False):
                for tcc in tcs:
                    asts = [st for st in range(ST) if act[br][st][tcc]]
                    if not asts:
                        for hp in range(2):
                            nc.vector.memset(
                                attnT[br * 2 + hp][:, tcc * 512 : (tcc + 1) * 512], 0.0
                            )
                        continue
                    eps_here = need_eps[br][tcc]
                    for hp in range(2):
                        qf = qkr[4 * br + hp]
                        kf = qkr[4 * br + 2 + hp]
                        # per-head [65, 512] AV banks: rows 0:64 = feats, 64 = sum;
                        # freed by this pass's normalize, reused by the next pass
                        avs = [
                            pat.tile(
                                [65, 512], f32, tag=f"av{ab}", bufs=1,
                                name=f"av{br}_{hp}_{ab}_{tcc}",
                            )
                            for ab in range(2)
                        ]
                        for st in asts:
                            pop_fill(1 + (len(fillq) > 20))
                            first = st == asts[0]
                            last = st == asts[-1]
                            needmul = mul[br][st][tcc]
                            # paired scores for both heads of this hp in 2 banks
                            scp = pat.tile(
                                [128, 1024], f32, tag="scp", bufs=2,
                                name=f"scp{br}_{hp}_{st}_{tcc}",
                            )
                            for ab in range(2):
                                nc.tensor.matmul(
                                    scp[:, ab * 512 : (ab + 1) * 512],
                                    kf[ab * 64 : ab * 64 + 64, st * 128 : (st + 1) * 128],
                                    qf[ab * 64 : ab * 64 + 64, tcc * 512 : (tcc + 1) * 512],
                                    start=True,
                                    stop=True,
                                    tile_position=(ab * 64, 0),
                                )
                            em = empool.tile(
                                [128, 1024], bf, tag="em", bufs=3,
                                name=f"em{br}_{hp}_{st}_{tcc}",
                            )
                            nc.scalar.activation(em[:], scp[:], Act.Exp, bias=expbias[br])
                            if needmul:
                                key = (br, tcc, st)
                                if key not in mask_idx:
                                    mask_idx[key] = mulctr[0]
                                    mulctr[0] += 1
                                mk = mpool.tile(
                                    [128, 512], bf, tag="mk", name=f"mk{br}_{st}_{tcc}_{hp}"
                                )
                                nc.sync.dma_start(mk[:], mask_d[mask_idx[key]])
                                mrep = mk[:].unsqueeze(1).to_broadcast([128, 2, 512])
                                emv = em[:].rearrange("p (r f) -> p r f", r=2)
                                eng = nc.vector if (st + hp) % 2 else nc.gpsimd
                                eng.tensor_tensor(emv, emv, mrep, Alu.mult)
                            for ab in range(2):
                                g = hp * 2 + ab
                                nc.tensor.matmul(
                                    avs[ab][:],
                                    v_sb[st][:, (br * 4 + g) * 65 : (br * 4 + g) * 65 + 65],
                                    em[:, ab * 512 : (ab + 1) * 512],
                                    start=first,
                                    stop=(last and not eps_here),
                                    tile_position=(0, 0),
                                )
                        if eps_here:
                            for ab in range(2):
                                nc.tensor.matmul(
                                    avs[ab][:],
                                    epsv[:],
                                    ones512[:],
                                    start=False,
                                    stop=True,
                                    tile_position=(0, 0),
                                )
                        # normalize: attnT = av[0:64] * (1/av[64]) bcast to 64 rows
                        # (gpsimd can't touch PSUM; DVE TT allows only one PSUM
                        # input: broadcast 1/sums into SBUF on Pool, mult on DVE)
                        for ab in range(2):
                            g = hp * 2 + ab
                            rcp = work.tile(
                                [1, 512], f32, tag="rcp", bufs=4,
                                name=f"rcp{br}_{g}_{tcc}",
                            )
                            nc.vector.reciprocal(rcp[:], avs[ab][64:65, :])
                            rb_sb = work.tile(
                                [64, 512], f32, tag="rb", bufs=4, name=f"rb{br}_{g}_{tcc}"
                            )
                            nc.gpsimd.partition_broadcast(rb_sb[:], rcp[:])
                            nc.vector.tensor_tensor(
                                attnT[br * 2 + hp][
                                    ab * 64 : ab * 64 + 64, tcc * 512 : (tcc + 1) * 512
                                ],
                                avs[ab][0:64, :],
                                rb_sb[:],
                                Alu.mult,
                            )
                    if emit_po:
                        for et in range(8):
                            fillq.append(lambda et=et, tcc=tcc: outproj_tile(et, tcc))

            # ---- emission order chosen for cross-phase engine overlap:
            # k/q intra proj+rope and v_intra first (PE), then intra attention
            # (ACT-heavy) while inter projections fill PE, then inter attention
            # overlapping the output projection.
            pat_holder = [None]
            for _rep in range(reps):
                if _rep == 0:
                    # all inter-branch q/k eagerly (attention(1) reads them at
                    # its first st iteration with st-outer loops); v_inter
                    # chunks interleave to fill PE gaps while rope runs on DVE
                    for fi, ft in enumerate((6, 4, 7, 5)):
                        qk_proj(ft, pq, "pqs")
                        for st in range(4 * fi, 4 * fi + 4):
                            _v_chunk(st, 1, pq, "vfill", 2)
                    pq_ctx.close()  # release early psum banks
                    pat_holder[0] = pat_ctx.enter_context(
                        tc.tile_pool(name="pat", bufs=1, space="PSUM")
                    )
                else:
                    for ft in (6, 4, 7, 5):
                        qk_proj(ft, pat_holder[0], "fill")
                pat = pat_holder[0]
                # braid the remaining projections into the ACT-heavy inter
                # branch; the light intra branch runs last with outproj braided.
                for ft in (2, 0, 3, 1):
                    fillq.extend(qk_proj_tasks(ft, pat, "fill"))
                fillq.extend(v_proj_tasks(0, pat, "fill"))
                attention(1, pat, list(range(TC)))
                drain_fill()  # all qkr/v writes emitted before intra readers
                attention(0, pat, list(range(TC)), emit_po=True)
                mask_idx.clear()
                mulctr[0] = 0

            drain_fill()
            pat_ctx.close()

    nc.finalize()
    return nc


# ------------------------------------------------------------------- kernel
def _kernel_prep(**inputs):
    hs = np.asarray(inputs["hidden_states"], np.float32)
    lens = np.asarray(inputs["attention_mask_in_length"])
    Wqkv = [np.asarray(inputs["Wqkv_intra"], np.float32), np.asarray(inputs["Wqkv_inter"], np.float32)]
    bqkv = [np.asarray(inputs["bqkv_intra"], np.float32), np.asarray(inputs["bqkv_inter"], np.float32)]
    Wout = np.asarray(inputs["Wout"], np.float32)
    bout = np.asarray(inputs["bout"], np.float32)

    # chain info + masks per batch
    pos_b, masks_b, rope_b = [], [], []
    for b in range(B):
        pos, cid, valid = _chain_info_np(lens[b])
        mi, mx = _masks_np(cid, valid)
        masks_b.append((mi, mx))
        rope_b.append(_rope_tables_fm(pos))

    # union tile activity across batches (single SPMD program)
    act = [[[False] * TC for _ in range(ST)] for _ in range(2)]
    mul = [[[False] * TC for _ in range(ST)] for _ in range(2)]
    eps = [[False] * TC for _ in range(2)]
    for br in range(2):
        for st in range(ST):
            for tcc in range(TC):
                for b in range(B):
                    m = masks_b[b][br][st * 128 : (st + 1) * 128, tcc * 512 : (tcc + 1) * 512]
                    a = bool(m.any())
                    act[br][st][tcc] |= a
                    mul[br][st][tcc] |= not bool(m.all())
        for tcc in range(TC):
            for b in range(B):
                col = masks_b[b][br][:, tcc * 512 : (tcc + 1) * 512].astype(np.float32)
                eps[br][tcc] |= bool((col.sum(axis=0) == 0).any())

    # exp-overflow guard: estimate score bound from a sample of positions
    expbias = [0.0, 0.0]
    idx = np.linspace(0, L - 1, 128).astype(int)
    for br in range(2):
        mx = 0.0
        for b in range(B):
            xs = hs[b][idx]  # [128, D]
            qkv = xs @ Wqkv[br].T + bqkv[br]
            q = qkv[:, :D].reshape(128, H, HD) * (HD**-0.5)
            k = qkv[:, D : 2 * D].reshape(128, H, HD)
            s = np.einsum("thd,shd->hts", q, k)
            mx = max(mx, float(np.abs(s).max()))
        if mx * 2.0 > 60.0:
            expbias[br] = -mx * 1.5  # RoPE preserves norms; 1.5x margin

    packs = [[], []]  # per batch; order mirrors device emission (first-use)
    for br in (1, 0):
        for tcc in range(TC):
            for st in range(ST):
                if act[br][st][tcc] and mul[br][st][tcc]:
                    for b in range(B):
                        packs[b].append(
                            masks_b[b][br][st * 128 : (st + 1) * 128, tcc * 512 : (tcc + 1) * 512]
                        )
    nmul = len(packs[0])
    maskp_b = [
        np.stack(p).astype(BF16) if p else np.zeros((1, 128, 512), BF16) for p in packs
    ]

    meta = {"act": act, "mul": mul, "expbias": expbias, "nmul": nmul, "eps": eps}
    global LAST_META
    LAST_META = meta
    nc = _build_program(meta)
    return nc, _in_maps(inputs, masks_b, rope_b, maskp_b)


def _in_maps(inputs, masks_b, rope_b, maskp_b):
    hs = np.asarray(inputs["hidden_states"], np.float32)
    Wqkv = [np.asarray(inputs["Wqkv_intra"], np.float32), np.asarray(inputs["Wqkv_inter"], np.float32)]
    bqkv = [np.asarray(inputs["bqkv_intra"], np.float32), np.asarray(inputs["bqkv_inter"], np.float32)]
    Wout = np.asarray(inputs["Wout"], np.float32)
    bout = np.asarray(inputs["bout"], np.float32)

    in_maps = []
    for c in range(NCORES):
        b, g = divmod(c, 4)
        h0 = g * HPC
        qrows = lambda br: Wqkv[br][h0 * HD : (h0 + HPC) * HD]
        krows = lambda br: Wqkv[br][D + h0 * HD : D + (h0 + HPC) * HD]
        vrows = lambda br: Wqkv[br][2 * D + h0 * HD : 2 * D + (h0 + HPC) * HD]
        qb = lambda br: bqkv[br][h0 * HD : (h0 + HPC) * HD]
        kb = lambda br: bqkv[br][D + h0 * HD : D + (h0 + HPC) * HD]
        vb = lambda br: bqkv[br][2 * D + h0 * HD : 2 * D + (h0 + HPC) * HD]
        sc = HD**-0.5
        # [D_in, 1024]: q_intra(256,scaled) k_intra(256) q_inter k_inter
        wqk_full = np.concatenate(
            [qrows(0).T * sc, krows(0).T, qrows(1).T * sc, krows(1).T], axis=1
        )
        wv_full = np.concatenate([vrows(0).T, vrows(1).T], axis=1)  # [D_in, 512]
        bqk_full = np.concatenate([qb(0) * sc, kb(0), qb(1) * sc, kb(1)])  # [1024]
        bv_full = np.concatenate([vb(0), vb(1)])  # [512]
        woutT = Wout[:, h0 * HD : (h0 + HPC) * HD].T  # [256, 1024]
        bout_c = bout if g == 0 else np.zeros_like(bout)
        cos128, sinS128 = rope_b[b]
        in_maps.append(
            {
                "xt": np.ascontiguousarray(
                    hs[b].T.reshape(KC, 128, L)
                ).astype(BF16),
                "wqk": np.ascontiguousarray(wqk_full.reshape(KC, 128, 1024)).astype(BF16),
                "wv": np.ascontiguousarray(wv_full.reshape(KC, 128, 512)).astype(BF16),
                "wout": np.ascontiguousarray(woutT.reshape(2, 128, 1024)).astype(BF16),
                "bqk": np.ascontiguousarray(bqk_full.reshape(8, 128).T).astype(np.float32),
                "bv": np.broadcast_to(bv_full, (128, 512)).astype(np.float32),
                "bout_t": np.ascontiguousarray(bout_c.reshape(8, 128).T).astype(np.float32),
                "cosf": np.ascontiguousarray(cos128),
                "sinf": np.ascontiguousarray(sinS128),
                "maskp": maskp_b[b],
            }
        )

    return in_maps


def prepare(**inputs):
    """Build the specialized program and per-core inputs."""
    return _kernel_prep(**inputs)


def kernel(**inputs):
    nc, in_maps = _kernel_prep(**inputs)
    try:  # cost-model estimate of HW time (NTFF profiling unavailable via axon)
        from concourse.bass_interp import CoreSim

        _sim = CoreSim(nc, no_exec=True, publish_trace=False)
        _sim.event_loop()
        global LAST_SIM_NS
        LAST_SIM_NS = _sim.time
    except Exception:
        LAST_SIM_NS = None

    res = run_bass_kernel_spmd(nc, in_maps, list(range(NCORES)))

    out = np.zeros((B, L, D), np.float32)
    for c in range(NCORES):
        b = c // 4
        ot = res.results[c]["outt"].reshape(D, L)  # [e, t]
        out[b] += ot.T
    return out


if __name__ == "__main__":
    rng = np.random.default_rng(0)
    import reference

    inputs = {k: np.asarray(v) for k, v in reference.setup_inputs().items()}
    got = kernel(**inputs)
    exp = np.asarray(reference.reference(**inputs))
    err = np.abs(got - exp).max() / np.abs(exp).max()
    print("rel err", err)



# revision 21
# speedup vs baseline: 1.0384x; 1.0176x over previous
"""Trainium2 Bass kernel for DecoupledIntraInterMultiSequenceSelfAttention.

Sharding: 8 cores = (batch in {0,1}) x (head-group in {0..3}, 4 heads each).
Each core computes both branches (intra/inter) for its batch/heads, plus the
partial output projection over its heads' columns of Wout. Host sums the 4
partial [D, L] outputs per batch and transposes back.

Device layout: feature-major ("transposed") throughout.
  - q,k produced as qkT [f, t] tiles (f on partitions), RoPE applied there.
  - scoresT[s, t] = k_aug.T @ q_aug per head (s on PSUM partitions).
  - em = exp(scoresT) -> bf16 sbuf; masked by 0/1 bf16 chain masks.
  - AV with fused softmax sums: stationary lhsT = [v_head | ones] (M=65),
    so avT[0:64] = sum_s v[s,d] em[s,t] and avT[64] = sum_s em[s,t] in the
    same matmul (per-head [65,512] PSUM bank).
  - normalize: reciprocal of the sums row, broadcast to 64 partitions via a
    K=1 matmul, per-head multiply -> attnT bf16 [d, t].
  - out projection: outT[e, t] += WoutT_chunk.T @ attnT_chunk, bias fused.
"""

import sys

sys.path.insert(0, "/opt/trn_rl_repo")

import numpy as np
import ml_dtypes

import concourse.bass as bass
import concourse.bacc as bacc
import concourse.tile as tile
from concourse import mybir
from concourse.bass_utils import run_bass_kernel_spmd

BF16 = ml_dtypes.bfloat16
LAST_SIM_NS = None
LAST_META = None
dt = mybir.dt
Alu = mybir.AluOpType
Act = mybir.ActivationFunctionType

B, L, D, H = 2, 2048, 1024, 16
HD = D // H
ROPE_BASE = 10000.0
NCORES = 8
HPC = 4  # heads per core
ST = L // 128  # 16 s-tiles
TC = L // 512  # 4 t-chunks
KC = D // 128  # 8 contraction chunks for projections


# ---------------------------------------------------------------- host math
def _chain_info_np(lengths_row):
    csum = np.cumsum(lengths_row.astype(np.int64))
    total = csum[-1]
    p = np.arange(L)
    cid = np.searchsorted(csum, p, side="right")
    prev = np.where(cid > 0, csum[np.clip(cid - 1, 0, L - 1)], 0)
    valid = p < total
    pos = np.where(valid, p - prev, 0).astype(np.float32)
    return pos, cid.astype(np.int64), valid


def _rope_tables_fm(pos):
    # feature-major cos/sin tables [128, L] (64-row pattern replicated x2),
    # with the rotation sign baked into sin ("sinS").
    inv_freq = 1.0 / (ROPE_BASE ** (np.arange(0, HD, 2, dtype=np.float32) / HD))
    # row d (0..63): frequency inv_freq[d % 32]
    freqs = pos[None, :] * inv_freq[(np.arange(HD) % (HD // 2)), None]  # [64, L]
    cos = np.cos(freqs)
    sin = np.sin(freqs)
    sinS = sin.copy()
    sinS[: HD // 2] *= -1.0  # rot(x)[d] = -x[d+32] for d<32 ; +x[d-32] else
    cos128 = np.concatenate([cos, cos], axis=0)
    sinS128 = np.concatenate([sinS, sinS], axis=0)
    return cos128.astype(BF16), sinS128.astype(BF16)


def _masks_np(cid, valid):
    cid_k = np.where(valid, cid, -5)  # sentinel for invalid keys
    eq = cid_k[:, None] == cid[None, :]  # [s, t] same-chain & key-valid
    intra = eq & valid[:, None]
    inter = valid[:, None] & ~eq
    return intra.astype(BF16), inter.astype(BF16)


# ------------------------------------------------------------- program build
def _build_program(meta, reps=1):
    """meta: dict with act[br][st][tc], mul[br][st][tc], expbias[br], eps[br][tc]"""
    act = meta["act"]
    mul = meta["mul"]
    expbias = meta["expbias"]
    need_eps = meta["eps"]

    nc = bacc.Bacc(None, target_bir_lowering=False)
    f32, bf = dt.float32, dt.bfloat16

    xt = nc.declare_dram_parameter("xt", [KC, 128, L], bf, isOutput=False)
    wqk = nc.declare_dram_parameter("wqk", [KC, 128, 1024], bf, isOutput=False)
    wv = nc.declare_dram_parameter("wv", [KC, 128, 512], bf, isOutput=False)
    wout = nc.declare_dram_parameter("wout", [2, 128, 1024], bf, isOutput=False)
    bqk = nc.declare_dram_parameter("bqk", [128, 8], f32, isOutput=False)
    bv = nc.declare_dram_parameter("bv", [128, 512], f32, isOutput=False)
    boutp = nc.declare_dram_parameter("bout_t", [128, 8], f32, isOutput=False)
    cosf = nc.declare_dram_parameter("cosf", [128, L], bf, isOutput=False)
    sinf = nc.declare_dram_parameter("sinf", [128, L], bf, isOutput=False)
    nmul = max(1, meta["nmul"])
    mask_d = nc.declare_dram_parameter("maskp", [nmul, 128, 512], bf, isOutput=False)
    outt = nc.declare_dram_parameter("outt", [8, 128, L], f32, isOutput=True)

    with tile.TileContext(nc) as tc:
        import contextlib

        ctx = contextlib.ExitStack()
        with ctx:
            persist = ctx.enter_context(tc.tile_pool(name="persist", bufs=1))
            work = ctx.enter_context(tc.tile_pool(name="work", bufs=2))
            empool = ctx.enter_context(tc.tile_pool(name="em", bufs=1))
            mpool = ctx.enter_context(tc.tile_pool(name="mask", bufs=8))
            pq_ctx = contextlib.ExitStack()
            pq = pq_ctx.enter_context(tc.tile_pool(name="pq", bufs=2, space="PSUM"))

            # ---- persistent sbuf tiles
            xt_sb = [persist.tile([128, L], bf, tag=f"xt{k}", name=f"xt{k}") for k in range(KC)]
            wqk_sb = [persist.tile([128, 1024], bf, tag=f"wqk{k}", name=f"wqk{k}") for k in range(KC)]
            wv_sb = [persist.tile([128, 512], bf, tag=f"wv{k}", name=f"wv{k}") for k in range(KC)]
            wout_sb = [persist.tile([128, 1024], bf, tag=f"wo{k}", name=f"wo{k}") for k in range(2)]
            cos_sb = persist.tile([128, L], bf, tag="cos", name="cos_sb")
            sin_sb = persist.tile([128, L], bf, tag="sin", name="sin_sb")
            bqk_sb = persist.tile([128, 8], f32, tag="bqk", name="bqk_sb")
            bo_sb = persist.tile([128, 8], f32, tag="bo", name="bo_sb")
            bv_bc = persist.tile([128, 512], f32, tag="bvb", name="bv_bc")
            ones512 = persist.tile([1, 512], bf, tag="ones512", name="ones512")
            epsv = persist.tile([1, 65], bf, tag="epsv", name="epsv")
            # per-head blocks of 65 cols: [v features (64) | ones] x 8 heads
            v_sb = [persist.tile([128, 520], bf, tag=f"v{s}", name=f"v{s}") for s in range(ST)]
            qkr = [persist.tile([128, L], bf, tag=f"qkr{i}", name=f"qkr{i}") for i in range(8)]
            attnT = [
                persist.tile([128, L], bf, tag=f"at{i}", name=f"at{i}") for i in range(4)
            ]  # [br*2+hp]

            for k in range(KC):
                (nc.scalar if k % 2 else nc.sync).dma_start(xt_sb[k][:], xt[k])
                # inter-branch weight columns first: the early phase only needs them
                (nc.sync if k % 2 else nc.scalar).dma_start(
                    wqk_sb[k][:, 512:1024], wqk[k, :, 512:1024]
                )
                nc.gpsimd.dma_start(wv_sb[k][:], wv[k])
            for k in range(KC):
                (nc.sync if k % 2 else nc.scalar).dma_start(
                    wqk_sb[k][:, 0:512], wqk[k, :, 0:512]
                )
            for k in range(2):
                nc.gpsimd.dma_start(wout_sb[k][:], wout[k])
            nc.gpsimd.dma_start(cos_sb[:], cosf[:])
            nc.gpsimd.dma_start(sin_sb[:], sinf[:])
            nc.gpsimd.dma_start(bqk_sb[:], bqk[:])
            nc.gpsimd.dma_start(bo_sb[:], boutp[:])
            nc.gpsimd.dma_start(bv_bc[:], bv[:])
            nc.vector.memset(ones512[:], 1.0)
            nc.vector.memset(epsv[:], 0.0)
            nc.vector.memset(epsv[:, 64:65], 1e-30)
            for st in range(ST):
                # ones column per head block (col 64 of each 65-wide block)
                nc.gpsimd.memset(
                    v_sb[st][:].rearrange("p (g c) -> p g c", g=8)[:, :, 64:65], 1.0
                )
            # preload the Exp activation table during the projection phase
            actwarm = work.tile([1, 8], f32, tag="actwarm", bufs=1, name="actwarm")
            nc.scalar.activation(actwarm[:], ones512[:, 0:8], Act.Exp, bias=expbias[1])

            # shared psum pool for attention + late projections (created after
            # the early-phase pq pool is released; budget: scp 2x2 + av 3 + fill 1 = 8)
            pat_ctx = contextlib.ExitStack()

            def _rope(ft, qk_t):
                # RoPE: qkr = qk*cos + rot(qk)*sinS  (rot = partition swap +-32)
                rot = work.tile([128, L], bf, tag="rot", bufs=2, name=f"rot{ft}")
                for h in range(4):
                    src_p = (h ^ 1) * 32
                    nc.vector.tensor_copy(
                        rot[h * 32 : h * 32 + 32, :], qk_t[src_p : src_p + 32, :]
                    )
                tmp = work.tile([128, L], bf, tag="ropetmp", bufs=2, name=f"rtmp{ft}")
                nc.vector.tensor_tensor(tmp[:], qk_t[:], cos_sb[:], Alu.mult)
                nc.gpsimd.tensor_tensor(rot[:], rot[:], sin_sb[:], Alu.mult)
                nc.vector.tensor_tensor(qkr[ft][:], tmp[:], rot[:], Alu.add)

            def _qk_chunk(ft, tp, grp, qk_t, psum_pool, ptag, bufs):
                pr = [
                    psum_pool.tile(
                        [128, 512], f32, tag=ptag, bufs=bufs,
                        name=f"qkps{ft}_{tp}_{j}",
                    )
                    for j in range(grp)
                ]
                for k in range(KC):
                    for j in range(grp):
                        tcc = tp * grp + j
                        nc.tensor.matmul(
                            pr[j][:],
                            wqk_sb[k][:, ft * 128 : (ft + 1) * 128],
                            xt_sb[k][:, tcc * 512 : (tcc + 1) * 512],
                            start=(k == 0),
                            stop=(k == KC - 1),
                        )
                for j in range(grp):
                    tcc = tp * grp + j
                    nc.vector.tensor_scalar(
                        qk_t[:, tcc * 512 : (tcc + 1) * 512],
                        pr[j][:],
                        bqk_sb[:, ft : ft + 1],
                        None,
                        Alu.add,
                    )

            def qk_proj(ft, psum_pool, ptag):
                # tcc-paired in the 2-slot early pool so each ldweights(wqk
                # chunk) serves 2 matmuls; single-slot pools go tcc-sequential.
                qk_t = work.tile([128, L], bf, tag="qk", bufs=2, name=f"qk{ft}")
                grp = 2 if ptag == "pqs" else 1
                bufs = 2
                for tp in range(TC // grp):
                    _qk_chunk(ft, tp, grp, qk_t, psum_pool, ptag, bufs)
                _rope(ft, qk_t)

            def qk_proj_tasks(ft, psum_pool, ptag):
                # fill-task closures: one tcc per task + a final rope task
                qk_t = work.tile([128, L], bf, tag="qk", bufs=2, name=f"qk{ft}")
                tasks = [
                    (lambda ft=ft, tp=tp, qk_t=qk_t: _qk_chunk(
                        ft, tp, 1, qk_t, psum_pool, ptag, 1))
                    for tp in range(TC)
                ]
                tasks.append(lambda ft=ft, qk_t=qk_t: _rope(ft, qk_t))
                return tasks

            def _v_chunk(st, br, psum_pool, ptag, bufs):
                ps = psum_pool.tile(
                    [128, 256], f32, tag=ptag, bufs=bufs, name=f"vps{br}_{st}"
                )
                for k in range(KC):
                    nc.tensor.matmul(
                        ps[:],
                        xt_sb[k][:, st * 128 : (st + 1) * 128],
                        wv_sb[k][:, br * 256 : (br + 1) * 256],
                        start=(k == 0),
                        stop=(k == KC - 1),
                    )
                vout = v_sb[st][:, br * 260 : (br + 1) * 260].rearrange(
                    "p (g c) -> p g c", g=4
                )[:, :, 0:64]
                nc.vector.scalar_tensor_tensor(
                    vout,
                    ps[:].rearrange("p (g c) -> p g c", g=4),
                    0.0,
                    bv_bc[:, br * 256 : (br + 1) * 256].rearrange("p (g c) -> p g c", g=4),
                    Alu.bypass,
                    Alu.add,
                )

            def v_proj_tasks(br, psum_pool, ptag):
                return [
                    (lambda st=st: _v_chunk(st, br, psum_pool, ptag, 1))
                    for st in range(ST)
                ]

            fillq = []

            def pop_fill(n=1):
                for _ in range(n):
                    if fillq:
                        fillq.pop(0)()

            def drain_fill():
                while fillq:
                    fillq.pop(0)()

            def outproj_tile(et, tcc, wide=False):
                tags = ("fill", "av0", "av1", "av2") if wide else ("fill",)
                po = pat_holder[0].tile(
                    [128, 512], f32, tag=tags[et % len(tags)], bufs=1,
                    name=f"po{et}_{tcc}",
                )
                n = 0
                for br in range(2):
                    for hp in range(2):
                        nc.tensor.matmul(
                            po[:],
                            wout_sb[hp][:, et * 128 : (et + 1) * 128],
                            attnT[br * 2 + hp][:, tcc * 512 : (tcc + 1) * 512],
                            start=(n == 0),
                            stop=(n == 3),
                        )
                        n += 1
                osb = work.tile([128, 512], f32, tag="osb", bufs=3, name=f"osb{et}_{tcc}")
                nc.vector.tensor_scalar(
                    osb[:], po[:], bo_sb[:, et : et + 1], None, Alu.add
                )
                nc.sync.dma_start(outt[et, :, tcc * 512 : (tcc + 1) * 512], osb[:])

            mulctr = [0]
            mask_idx = {}

            passctr = [0]

            def attention(br, pat, tcs, emit_po=False):
                # hp is outermost: the hp0 sweep only needs qkr[4br],[4br+2],
                # so the other head-pair's rope can still be in flight.
                for hp in range(2):
                    qf = qkr[4 * br + hp]
                    kf = qkr[4 * br + 2 + hp]
                    # mask prefetch order for this sweep
                    mseq = [
                        (tcc, st)
                        for tcc in tcs
                        for st in range(ST)
                        if act[br][st][tcc] and mul[br][st][tcc]
                    ]
                    mtiles = {}
                    mnext = [0]

                    def mask_prefetch():
                        if mnext[0] < len(mseq):
                            tcc, st = mseq[mnext[0]]
                            mnext[0] += 1
                            key = (br, tcc, st)
                            if key not in mask_idx:
                                mask_idx[key] = mulctr[0]
                                mulctr[0] += 1
                            mk = mpool.tile(
                                [128, 512], bf, tag="mk", name=f"mk{br}_{st}_{tcc}_{hp}"
                            )
                            nc.sync.dma_start(mk[:], mask_d[mask_idx[key]])
                            mtiles[(tcc, st)] = mk

                    mask_prefetch()
                    mask_prefetch()
                    for tcc in tcs:
                        asts = [st for st in range(ST) if act[br][st][tcc]]
                        if not asts:
                            nc.vector.memset(
                                attnT[br * 2 + hp][:, tcc * 512 : (tcc + 1) * 512], 0.0
                            )
                            continue
                        eps_here = need_eps[br][tcc]
                        # per-head [65, 512] AV banks (rows 0:64 feats, 64 sum)
                        # rotate over 3 tags so the next pass never waits on
                        # this pass's normalize chain
                        avs = [
                            pat.tile(
                                [65, 512], f32, tag=f"av{(2 * passctr[0] + ab) % 3}",
                                bufs=1, name=f"av{br}_{hp}_{ab}_{tcc}",
                            )
                            for ab in range(2)
                        ]
                        passctr[0] += 1

                        def emit_av(st, em, first, last):
                            for ab in range(2):
                                g = hp * 2 + ab
                                nc.tensor.matmul(
                                    avs[ab][:],
                                    v_sb[st][:, (br * 4 + g) * 65 : (br * 4 + g) * 65 + 65],
                                    em[:, ab * 512 : (ab + 1) * 512],
                                    start=first,
                                    stop=(last and not eps_here),
                                    tile_position=(0, 0),
                                )

                        prev = None  # software-pipeline AV one st behind scores
                        for st in asts:
                            pop_fill(1 + (len(fillq) > 20))
                            needmul = mul[br][st][tcc]
                            # paired scores for both heads of this hp in 2 banks
                            scp = pat.tile(
                                [128, 1024], f32, tag="scp", bufs=2,
                                name=f"scp{br}_{hp}_{st}_{tcc}",
                            )
                            for ab in range(2):
                                nc.tensor.matmul(
                                    scp[:, ab * 512 : (ab + 1) * 512],
                                    kf[ab * 64 : ab * 64 + 64, st * 128 : (st + 1) * 128],
                                    qf[ab * 64 : ab * 64 + 64, tcc * 512 : (tcc + 1) * 512],
                                    start=True,
                                    stop=True,
                                    tile_position=(ab * 64, 0),
                                )
                            em = empool.tile(
                                [128, 1024], bf, tag="em", bufs=3,
                                name=f"em{br}_{hp}_{st}_{tcc}",
                            )
                            nc.scalar.activation(em[:], scp[:], Act.Exp, bias=expbias[br])
                            if needmul:
                                mk = mtiles.pop((tcc, st))
                                mask_prefetch()
                                mrep = mk[:].unsqueeze(1).to_broadcast([128, 2, 512])
                                emv = em[:].rearrange("p (r f) -> p r f", r=2)
                                eng = nc.vector if (st + hp) % 2 else nc.gpsimd
                                eng.tensor_tensor(emv, emv, mrep, Alu.mult)
                            if prev is not None:
                                emit_av(prev[0], prev[1], prev[0] == asts[0], False)
                            prev = (st, em)
                        emit_av(prev[0], prev[1], prev[0] == asts[0], True)
                        if eps_here:
                            for ab in range(2):
                                nc.tensor.matmul(
                                    avs[ab][:],
                                    epsv[:],
                                    ones512[:],
                                    start=False,
                                    stop=True,
                                    tile_position=(0, 0),
                                )
                        # normalize: attnT = av[0:64] * (1/av[64]) bcast to 64
                        # rows (gpsimd can't touch PSUM; DVE TT allows only one
                        # PSUM input: bcast 1/sums into SBUF on Pool, mult DVE)
                        for ab in range(2):
                            g = hp * 2 + ab
                            rcp = work.tile(
                                [1, 512], f32, tag="rcp", bufs=4,
                                name=f"rcp{br}_{g}_{tcc}",
                            )
                            nc.vector.reciprocal(rcp[:], avs[ab][64:65, :])
                            rb_sb = work.tile(
                                [64, 512], f32, tag="rb", bufs=4, name=f"rb{br}_{g}_{tcc}"
                            )
                            nc.gpsimd.partition_broadcast(rb_sb[:], rcp[:])
                            nc.vector.tensor_tensor(
                                attnT[br * 2 + hp][
                                    ab * 64 : ab * 64 + 64, tcc * 512 : (tcc + 1) * 512
                                ],
                                avs[ab][0:64, :],
                                rb_sb[:],
                                Alu.mult,
                            )
                        if emit_po and hp == 1:
                            wide = tcc == tcs[-1]
                            for et in range(8):
                                fillq.append(
                                    lambda et=et, tcc=tcc, wide=wide: outproj_tile(et, tcc, wide)
                                )

            # ---- emission order chosen for cross-phase engine overlap:
            # k/q intra proj+rope and v_intra first (PE), then intra attention
            # (ACT-heavy) while inter projections fill PE, then inter attention
            # overlapping the output projection.
            pat_holder = [None]
            for _rep in range(reps):
                if _rep == 0:
                    # all inter-branch q/k eagerly (attention(1) reads them at
                    # its first st iteration with st-outer loops); v_inter
                    # chunks interleave to fill PE gaps while rope runs on DVE
                    for fi, ft in enumerate((6, 4, 7, 5)):
                        qk_proj(ft, pq, "pqs")
                        for st in range(4 * fi, 4 * fi + 4):
                            _v_chunk(st, 1, pq, "vfill", 2)
                    pq_ctx.close()  # release early psum banks
                    pat_holder[0] = pat_ctx.enter_context(
                        tc.tile_pool(name="pat", bufs=1, space="PSUM")
                    )
                else:
                    for ft in (6, 4, 7, 5):
                        qk_proj(ft, pat_holder[0], "fill")
                pat = pat_holder[0]
                # braid the remaining projections into the ACT-heavy inter
                # branch; the light intra branch runs last with outproj braided.
                for ft in (2, 0, 3, 1):
                    fillq.extend(qk_proj_tasks(ft, pat, "fill"))
                fillq.extend(v_proj_tasks(0, pat, "fill"))
                attention(1, pat, list(range(TC)))
                drain_fill()  # all qkr/v writes emitted before intra readers
                attention(0, pat, list(range(TC)), emit_po=True)
                mask_idx.clear()
                mulctr[0] = 0

            drain_fill()
            pat_ctx.close()

    nc.finalize()
    return nc


# ------------------------------------------------------------------- kernel
def _kernel_prep(**inputs):
    hs = np.asarray(inputs["hidden_states"], np.float32)
    lens = np.asarray(inputs["attention_mask_in_length"])
    Wqkv = [np.asarray(inputs["Wqkv_intra"], np.float32), np.asarray(inputs["Wqkv_inter"], np.float32)]
    bqkv = [np.asarray(inputs["bqkv_intra"], np.float32), np.asarray(inputs["bqkv_inter"], np.float32)]
    Wout = np.asarray(inputs["Wout"], np.float32)
    bout = np.asarray(inputs["bout"], np.float32)

    # chain info + masks per batch
    pos_b, masks_b, rope_b = [], [], []
    for b in range(B):
        pos, cid, valid = _chain_info_np(lens[b])
        mi, mx = _masks_np(cid, valid)
        masks_b.append((mi, mx))
        rope_b.append(_rope_tables_fm(pos))

    # union tile activity across batches (single SPMD program)
    act = [[[False] * TC for _ in range(ST)] for _ in range(2)]
    mul = [[[False] * TC for _ in range(ST)] for _ in range(2)]
    eps = [[False] * TC for _ in range(2)]
    for br in range(2):
        for st in range(ST):
            for tcc in range(TC):
                for b in range(B):
                    m = masks_b[b][br][st * 128 : (st + 1) * 128, tcc * 512 : (tcc + 1) * 512]
                    a = bool(m.any())
                    act[br][st][tcc] |= a
                    mul[br][st][tcc] |= not bool(m.all())
        for tcc in range(TC):
            for b in range(B):
                col = masks_b[b][br][:, tcc * 512 : (tcc + 1) * 512].astype(np.float32)
                eps[br][tcc] |= bool((col.sum(axis=0) == 0).any())

    # exp-overflow guard: estimate score bound from a sample of positions
    expbias = [0.0, 0.0]
    idx = np.linspace(0, L - 1, 128).astype(int)
    for br in range(2):
        mx = 0.0
        for b in range(B):
            xs = hs[b][idx]  # [128, D]
            qkv = xs @ Wqkv[br].T + bqkv[br]
            q = qkv[:, :D].reshape(128, H, HD) * (HD**-0.5)
            k = qkv[:, D : 2 * D].reshape(128, H, HD)
            s = np.einsum("thd,shd->hts", q, k)
            mx = max(mx, float(np.abs(s).max()))
        if mx * 2.0 > 60.0:
            expbias[br] = -mx * 1.5  # RoPE preserves norms; 1.5x margin

    packs = [[], []]  # per batch; order mirrors device emission (first-use)
    for br in (1, 0):
        for tcc in range(TC):
            for st in range(ST):
                if act[br][st][tcc] and mul[br][st][tcc]:
                    for b in range(B):
                        packs[b].append(
                            masks_b[b][br][st * 128 : (st + 1) * 128, tcc * 512 : (tcc + 1) * 512]
                        )
    nmul = len(packs[0])
    maskp_b = [
        np.stack(p).astype(BF16) if p else np.zeros((1, 128, 512), BF16) for p in packs
    ]

    meta = {"act": act, "mul": mul, "expbias": expbias, "nmul": nmul, "eps": eps}
    global LAST_META
    LAST_META = meta
    nc = _build_program(meta)
    return nc, _in_maps(inputs, masks_b, rope_b, maskp_b)


def _in_maps(inputs, masks_b, rope_b, maskp_b):
    hs = np.asarray(inputs["hidden_states"], np.float32)
    Wqkv = [np.asarray(inputs["Wqkv_intra"], np.float32), np.asarray(inputs["Wqkv_inter"], np.float32)]
    bqkv = [np.asarray(inputs["bqkv_intra"], np.float32), np.asarray(inputs["bqkv_inter"], np.float32)]
    Wout = np.asarray(inputs["Wout"], np.float32)
    bout = np.asarray(inputs["bout"], np.float32)

    in_maps = []
    for c in range(NCORES):
        b, g = divmod(c, 4)
        h0 = g * HPC
        qrows = lambda br: Wqkv[br][h0 * HD : (h0 + HPC) * HD]
        krows = lambda br: Wqkv[br][D + h0 * HD : D + (h0 + HPC) * HD]
        vrows = lambda br: Wqkv[br][2 * D + h0 * HD : 2 * D + (h0 + HPC) * HD]
        qb = lambda br: bqkv[br][h0 * HD : (h0 + HPC) * HD]
        kb = lambda br: bqkv[br][D + h0 * HD : D + (h0 + HPC) * HD]
        vb = lambda br: bqkv[br][2 * D + h0 * HD : 2 * D + (h0 + HPC) * HD]
        sc = HD**-0.5
        # [D_in, 1024]: q_intra(256,scaled) k_intra(256) q_inter k_inter
        wqk_full = np.concatenate(
            [qrows(0).T * sc, krows(0).T, qrows(1).T * sc, krows(1).T], axis=1
        )
        wv_full = np.concatenate([vrows(0).T, vrows(1).T], axis=1)  # [D_in, 512]
        bqk_full = np.concatenate([qb(0) * sc, kb(0), qb(1) * sc, kb(1)])  # [1024]
        bv_full = np.concatenate([vb(0), vb(1)])  # [512]
        woutT = Wout[:, h0 * HD : (h0 + HPC) * HD].T  # [256, 1024]
        bout_c = bout if g == 0 else np.zeros_like(bout)
        cos128, sinS128 = rope_b[b]
        in_maps.append(
            {
                "xt": np.ascontiguousarray(
                    hs[b].T.reshape(KC, 128, L)
                ).astype(BF16),
                "wqk": np.ascontiguousarray(wqk_full.reshape(KC, 128, 1024)).astype(BF16),
                "wv": np.ascontiguousarray(wv_full.reshape(KC, 128, 512)).astype(BF16),
                "wout": np.ascontiguousarray(woutT.reshape(2, 128, 1024)).astype(BF16),
                "bqk": np.ascontiguousarray(bqk_full.reshape(8, 128).T).astype(np.float32),
                "bv": np.broadcast_to(bv_full, (128, 512)).astype(np.float32),
                "bout_t": np.ascontiguousarray(bout_c.reshape(8, 128).T).astype(np.float32),
                "cosf": np.ascontiguousarray(cos128),
                "sinf": np.ascontiguousarray(sinS128),
                "maskp": maskp_b[b],
            }
        )

    return in_maps


def prepare(**inputs):
    """Build the specialized program and per-core inputs."""
    return _kernel_prep(**inputs)


def kernel(**inputs):
    nc, in_maps = _kernel_prep(**inputs)
    try:  # cost-model estimate of HW time (NTFF profiling unavailable via axon)
        from concourse.bass_interp import CoreSim

        _sim = CoreSim(nc, no_exec=True, publish_trace=False)
        _sim.event_loop()
        global LAST_SIM_NS
        LAST_SIM_NS = _sim.time
    except Exception:
        LAST_SIM_NS = None

    res = run_bass_kernel_spmd(nc, in_maps, list(range(NCORES)))

    out = np.zeros((B, L, D), np.float32)
    for c in range(NCORES):
        b = c // 4
        ot = res.results[c]["outt"].reshape(D, L)  # [e, t]
        out[b] += ot.T
    return out


if __name__ == "__main__":
    rng = np.random.default_rng(0)
    import reference

    inputs = {k: np.asarray(v) for k, v in reference.setup_inputs().items()}
    got = kernel(**inputs)
    exp = np.asarray(reference.reference(**inputs))
    err = np.abs(got - exp).max() / np.abs(exp).max()
    print("rel err", err)

